# revision 21
# baseline (speedup 1.0000x reference)
"""Trainium2 Bass kernel for nn_Dilate: 7x7 all-ones conv (same padding) -> (y > 0) int32 mask.

Input  x: (16, 1, 1024, 1024) float32, weight: (1, 1, 7, 7) ones (values unused).
Output:   (16, 1, 1024, 1024) int32 in {0, 1}.

Per core (pure batch data-parallel, 2 images/core on 8 cores), the 2D box
sum is separated HORIZONTAL-first so each engine does exactly one pass per
tile and the whole thing pipelines at the input-DMA roofline:

  - Row-tiles: 128 input rows (incl. 3+3 halo) -> 122 output rows, 9/image.
  - x loads via HWDGE (sync ring) into rotating [128, 7+W+3] SBUF buffers
    whose 7 leading + 3 trailing columns are zeroed once at startup.
  - Horizontal 7-tap sum in ONE custom-DVE instruction (registered at import
    into concourse.dve_ops.OPS): h = scan(ADD, Src0 - Src1) over the padded
    buffer = running sum of (x[t] - x[t-7]) = sliding 7-window sum.  The
    custom uop runs the recurrence at full rate (~1.2us/tile vs 2.3us for
    the stock tensor_tensor_scan, which routes its state backward through
    the pipe and halves throughput).
  - Vertical 7-tap sum on TensorE: banded ones matrix [128,122] as lhsT,
    fp32r matmul (PE rounds f32r internally; h is bitcast, no cast op),
    2x 512-col matmuls -> PSUM [122, 1024].
  - Threshold on ScalarE straight out of PSUM: sigmoid(1e8*boxsum) + round
    -to-nearest int8 cast (decision boundary exactly at boxsum=0).
  - int8 masks leave via GpSimd SWDGE; the host widens to int32.

Engine budget/tile: DVE ~1.21us, ACT ~1.0us, PE ~0.7-1.9us, DMA-in ~1.45us.
"""

import numpy as np

import concourse.bacc as bacc
import concourse.mybir as mybir
import concourse.dve_ops as dve_ops
from concourse.dve_spec import Spec, Src0, Src1, AluOp, scan, lower, _has_src1
from concourse.dve_uop import DveOpSpec
from concourse.tile import TileContext
from concourse.bass_utils import run_bass_kernel_spmd

B, H, W = 16, 1024, 1024
NCORES = 8
PER_CORE = B // NCORES  # 2 images per core
R = 7
PAD = R // 2  # 3
P = 128             # SBUF partitions per tile (input rows incl. halo)
MOUT = P - (R - 1)  # 122 output rows per tile
NTILES = -(-H // MOUT)  # 9 row tiles per image

WIN = W + PAD       # scan length: h col t = boxsum for output col j = t - 3
WB = R + W + PAD    # x tile width incl. 7 leading + 3 trailing zero cols
HOFF = 13           # h write offset so the matmul rhs (HOFF+PAD) is 32B-aligned
HB = HOFF + WIN     # h tile width

SIG_SCALE = 1.0e8   # pre-scale for the sigmoid threshold trick
N_X = 10            # rotating once-zero-padded x buffers (DMA prefetch depth)


def _register_boxsum7():
    """Register the custom DVE op (idempotent): out = cumsum(in0 - in1)."""
    name = "BOXSUM7_ANT"
    for op in dve_ops.OPS:
        if op.name == name:
            return op
    spec = Spec(
        body=scan(AluOp.ADD, Src0 - Src1),
        reference=lambda in0, in1, s0, s1, imm2: np.cumsum(
            in0.astype(np.float32) - in1.astype(np.float32), axis=-1
        ).astype(np.float32),
    )
    row = dve_ops._CUSTOM_DVE_ROW_BASE + len(dve_ops.OPS)
    assert row < 0x20, "custom-DVE row space exhausted"
    shas = {}
    for ver in ("v3", "v4"):
        s = DveOpSpec(name=name, opcode=row, uops=lower(spec, ver=ver),
                      rd1_en=_has_src1(spec))
        shas[ver] = s.sha(ver)
    op = dve_ops.DveOp(name, spec, subdim=False, uops_sha=shas)
    dve_ops.OPS.append(op)
    dve_ops._SUB_OPCODE_FOR_NAME[name] = row
    dve_ops.CUSTOM_DVE_SPECS[name] = spec
    return op


def _band_matrices() -> np.ndarray:
    """bands[0]: t=0 (partition p = image row p, top clamp);
    bands[1]: interior (partition p = row o0-3+p);
    bands[2]: last tile (partition p = row H-128+p, bottom clamp).
    band[k, m] = 1 iff output row m sums input partition k.
    Padded to 128 columns so the DMA moves 512 B/partition (line rate)."""
    bands = np.zeros((3, P, P), dtype=np.float16)
    for m in range(MOUT):
        bands[0, max(0, m - PAD) : m + PAD + 1, m] = 1.0
        bands[1, m : m + R, m] = 1.0
    # last tile: outputs start at row H-48 = partition 80
    for m in range(48):
        bands[2, 80 + m - PAD : min(80 + m + PAD + 1, P), m] = 1.0
    return bands


def _build_program():
    boxsum7 = _register_boxsum7()

    nc = bacc.Bacc("TRN2")
    x_d = nc.dram_tensor("x", [PER_CORE, H, W], mybir.dt.float32, kind="ExternalInput")
    band_d = nc.dram_tensor("band", [3, P, P], mybir.dt.float16, kind="ExternalInput")
    y_d = nc.dram_tensor("y", [PER_CORE, H, W], mybir.dt.int8, kind="ExternalOutput")

    sig = mybir.ActivationFunctionType.Sigmoid
    f16 = mybir.dt.float16
    f32 = mybir.dt.float32

    with TileContext(nc) as tc:
        with (
            tc.tile_pool(name="const", bufs=1) as cpool,
            tc.tile_pool(name="hbuf", bufs=6) as hpool,
            tc.tile_pool(name="mask", bufs=6) as mpool,
            tc.tile_pool(name="psum", bufs=3, space="PSUM") as psum_pool,
            tc.tile_pool(name="psum_w", bufs=1, space="PSUM") as psum_w,
        ):
            band_ts = []
            for i in range(3):
                bt = cpool.tile([P, P], f16, tag=f"band{i}")
                nc.scalar.dma_start(out=bt[:], in_=band_d[i])
                band_ts.append(bt)

            # PE warm-up: ~20 tiny fp16 matmuls on a zeroed tile during the
            # (otherwise PE-idle) preamble/load window trip the HAM activity
            # monitor to K=8/8, so the real matmuls run at 2.4 GHz instead of
            # the cold 1.2 GHz default.  Body MM gaps (~1us) never span a
            # full 3.4us idle window, so the PE stays warm afterwards.
            wtile = cpool.tile([P, 64], f16, tag="warm")
            nc.gpsimd.memset(wtile[:, :], 0.0)
            warm_ps = psum_w.tile([64, 64], f32)
            for _ in range(20):
                nc.tensor.matmul(
                    warm_ps[:, :], wtile[:, 0:64], wtile[:, 0:64],
                    start=True, stop=True,
                )

            # Rotating x buffers with 7 leading and 3 trailing zero columns
            # (zeroed once; loads always write cols 7..7+W), so one scan of
            # length W+3 yields every output column incl. both edges.
            xsb = []
            for i in range(N_X):
                xt = cpool.tile([P, WB], f32, tag=f"xsb{i}")
                nc.gpsimd.memset(xt[:, 0:R], 0.0)
                nc.gpsimd.memset(xt[:, R + W : WB], 0.0)
                xsb.append(xt)

            # (band_idx, row_lo of the 128-row input slab, out_row, nvalid)
            tiles = []
            for img in range(PER_CORE):
                for t in range(NTILES):
                    o0 = t * MOUT
                    if t == 0:
                        lo = 0
                    elif t == NTILES - 1:
                        lo = H - P
                    else:
                        lo = o0 - PAD
                    nvalid = min(MOUT, H - o0)
                    tiles.append(
                        (0 if t == 0 else (2 if t == NTILES - 1 else 1),
                         img, lo, o0, nvalid)
                    )

            # Loads are emitted with a LOOKAHEAD lead over their consumers so
            # program order stays correct on the rotating buffers (load i+N_X
            # rewrites scan i's buffer, so it must be emitted AFTER scan i
            # and the lead must stay < N_X).
            #
            # Halo recycling: a tile's input slab overlaps the previous slab
            # (6 rows interior, 9 after the first tile, 83 before the last).
            # Those rows are copied SBUF->SBUF on the (idle) scalar ring and
            # only the new rows are read from HBM: 9.5 -> 8.4 MB per core.
            LOOKAHEAD = 8
            n_total = len(tiles)

            def emit_load(i):
                _, img, lo, _, _ = tiles[i]
                x_t = xsb[i % N_X]
                if i % NTILES == 0:
                    n_halo = 0
                else:
                    prev_lo = tiles[i - 1][2]
                    n_halo = P - (lo - prev_lo)
                    nc.scalar.dma_start(
                        out=x_t[0:n_halo, R : R + W],
                        in_=xsb[(i - 1) % N_X][lo - prev_lo : P, R : R + W],
                    )
                nc.sync.dma_start(
                    out=x_t[n_halo:P, R : R + W],
                    in_=x_d[img, lo + n_halo : lo + P, :],
                )

            for i in range(min(LOOKAHEAD, n_total)):
                emit_load(i)

            for i, (band_idx, img, lo, o0, nvalid) in enumerate(tiles):
                if i + LOOKAHEAD < n_total:
                    emit_load(i + LOOKAHEAD)
                x_t = xsb[i % N_X]

                # horizontal sliding 7-sum, one full-rate DVE instruction;
                # the scan state is fp32 internally and downcasts to fp16 on
                # write, so the 2-byte matmul (full-rate streaming, 1024-col
                # moving operand) gets its rhs with no extra cast op.
                h_t = hpool.tile([P, HB], f16)
                nc.vector._custom_dve(
                    boxsum7,
                    out=h_t[:, HOFF : HOFF + WIN],
                    in0=x_t[:, R : R + WIN],
                    in1=x_t[:, 0:WIN],
                )

                # vertical 7-sum: banded fp16 matmul -> 2D boxsum in PSUM
                # (2x 512-col MMs: a single MM's PSUM output is 1-bank max)
                v_ps = psum_pool.tile([MOUT, W], f32)
                bt = band_ts[band_idx]
                for j in range(2):
                    nc.tensor.matmul(
                        v_ps[:, j * 512 : (j + 1) * 512],
                        bt[:, 0:MOUT],
                        h_t[:, HOFF + PAD + j * 512 : HOFF + PAD + (j + 1) * 512],
                        start=True,
                        stop=True,
                    )

                # threshold straight from PSUM: mask = boxsum > 0 -> int8
                m_t = mpool.tile([P, W], mybir.dt.int8)
                nc.scalar.activation(
                    m_t[:MOUT, :], v_ps[:, :], sig, scale=SIG_SCALE,
                )

                # int8 SWDGE out
                nc.gpsimd.dma_start(
                    out=y_d[img, o0 : o0 + nvalid, :],
                    in_=m_t[0:nvalid, :],
                )

    nc.compile()
    return nc


_PROGRAM_CACHE = {}


def _get_program():
    if "nc" not in _PROGRAM_CACHE:
        _PROGRAM_CACHE["nc"] = _build_program()
    return _PROGRAM_CACHE["nc"]


def kernel(x, weight=None, **_unused):
    x = np.ascontiguousarray(np.asarray(x), dtype=np.float32)
    assert x.shape == (B, 1, H, W), x.shape
    xs = x.reshape(B, H, W)
    band = _band_matrices()

    nc = _get_program()
    in_maps = [
        {"x": np.ascontiguousarray(xs[c * PER_CORE : (c + 1) * PER_CORE]), "band": band}
        for c in range(NCORES)
    ]
    res = run_bass_kernel_spmd(nc, in_maps, core_ids=list(range(NCORES)))
    out = np.concatenate([r["y"] for r in res.results], axis=0)
    return out.reshape(B, 1, H, W).astype(np.int32)


# revision 24
# speedup vs baseline: 3.0396x; 3.0396x over previous
"""Trainium2 Bass kernel for nn_Dilate: 7x7 all-ones conv (same padding) -> (y > 0) int32 mask.

Input  x: (16, 1, 1024, 1024) float32, weight: (1, 1, 7, 7) ones (values unused).
Output:   (16, 1, 1024, 1024) int32 in {0, 1}.

Per core (pure batch data-parallel, 2 images/core on 8 cores), the 2D box
sum is separated HORIZONTAL-first so each engine does exactly one pass per
tile and the whole thing pipelines at the input-DMA roofline:

  - Row-tiles: 128 input rows (incl. 3+3 halo) -> 122 output rows, 9/image.
  - x loads via HWDGE (sync ring) into rotating [128, 7+W+3] SBUF buffers
    whose 7 leading + 3 trailing columns are zeroed once at startup.
  - Horizontal 7-tap sum in ONE custom-DVE instruction (registered at import
    into concourse.dve_ops.OPS): h = scan(ADD, Src0 - Src1) over the padded
    buffer = running sum of (x[t] - x[t-7]) = sliding 7-window sum.  The
    custom uop runs the recurrence at full rate (~1.2us/tile vs 2.3us for
    the stock tensor_tensor_scan, which routes its state backward through
    the pipe and halves throughput).
  - Vertical 7-tap sum on TensorE: banded ones matrix [128,122] as lhsT,
    fp32r matmul (PE rounds f32r internally; h is bitcast, no cast op),
    2x 512-col matmuls -> PSUM [122, 1024].
  - Threshold on ScalarE straight out of PSUM: sigmoid(1e8*boxsum) + round
    -to-nearest int8 cast (decision boundary exactly at boxsum=0).
  - int8 masks leave via GpSimd SWDGE; the host widens to int32.

Engine budget/tile: DVE ~1.21us, ACT ~1.0us, PE ~0.7-1.9us, DMA-in ~1.45us.
"""

import numpy as np

import concourse.bacc as bacc
import concourse.mybir as mybir
import concourse.dve_ops as dve_ops
from concourse.dve_spec import Spec, Src0, Src1, AluOp, scan, lower, _has_src1
from concourse.dve_uop import DveOpSpec
from concourse.tile import TileContext
from concourse.bass_utils import run_bass_kernel_spmd

B, H, W = 16, 1024, 1024
NCORES = 8
PER_CORE = B // NCORES  # 2 images per core
R = 7
PAD = R // 2  # 3
P = 128             # SBUF partitions per tile (input rows incl. halo)
MOUT = P - (R - 1)  # 122 output rows per tile
NTILES = -(-H // MOUT)  # 9 row tiles per image

WIN = W + PAD       # scan length: h col t = boxsum for output col j = t - 3
WB = R + W + PAD    # x tile width incl. 7 leading + 3 trailing zero cols
HOFF = 13           # h write offset so the matmul rhs (HOFF+PAD) is 32B-aligned
HB = HOFF + WIN     # h tile width

SIG_SCALE = 1.0e8   # pre-scale for the sigmoid threshold trick
N_X = 10            # rotating once-zero-padded x buffers (DMA prefetch depth)


def _register_boxsum7():
    """Register the custom DVE op (idempotent): out = cumsum(in0 - in1)."""
    name = "BOXSUM7_ANT"
    for op in dve_ops.OPS:
        if op.name == name:
            return op
    spec = Spec(
        body=scan(AluOp.ADD, Src0 - Src1),
        reference=lambda in0, in1, s0, s1, imm2: np.cumsum(
            in0.astype(np.float32) - in1.astype(np.float32), axis=-1
        ).astype(np.float32),
    )
    row = dve_ops._CUSTOM_DVE_ROW_BASE + len(dve_ops.OPS)
    assert row < 0x20, "custom-DVE row space exhausted"
    shas = {}
    for ver in ("v3", "v4"):
        s = DveOpSpec(name=name, opcode=row, uops=lower(spec, ver=ver),
                      rd1_en=_has_src1(spec))
        shas[ver] = s.sha(ver)
    op = dve_ops.DveOp(name, spec, subdim=False, uops_sha=shas)
    dve_ops.OPS.append(op)
    dve_ops._SUB_OPCODE_FOR_NAME[name] = row
    dve_ops.CUSTOM_DVE_SPECS[name] = spec
    return op


def _band_matrices() -> np.ndarray:
    """bands[0]: t=0 (partition p = image row p, top clamp);
    bands[1]: interior (partition p = row o0-3+p);
    bands[2]: last tile (partition p = row H-128+p, bottom clamp).
    band[k, m] = 1 iff output row m sums input partition k.
    Padded to 128 columns so the DMA moves 512 B/partition (line rate)."""
    bands = np.zeros((3, P, P), dtype=np.float16)
    for m in range(MOUT):
        bands[0, max(0, m - PAD) : m + PAD + 1, m] = 1.0
        bands[1, m : m + R, m] = 1.0
    # last tile: outputs start at row H-48 = partition 80
    for m in range(48):
        bands[2, 80 + m - PAD : min(80 + m + PAD + 1, P), m] = 1.0
    return bands


def _build_program():
    boxsum7 = _register_boxsum7()

    nc = bacc.Bacc("TRN2")
    x_d = nc.dram_tensor("x", [PER_CORE, H, W], mybir.dt.float32, kind="ExternalInput")
    band_d = nc.dram_tensor("band", [3, P, P], mybir.dt.float16, kind="ExternalInput")
    y_d = nc.dram_tensor("y", [PER_CORE, H, W], mybir.dt.int8, kind="ExternalOutput")

    sig = mybir.ActivationFunctionType.Sigmoid
    f16 = mybir.dt.float16
    f32 = mybir.dt.float32

    with TileContext(nc) as tc:
        with (
            tc.tile_pool(name="const", bufs=1) as cpool,
            tc.tile_pool(name="hbuf", bufs=6) as hpool,
            tc.tile_pool(name="mask", bufs=6) as mpool,
            tc.tile_pool(name="psum", bufs=4, space="PSUM") as psum_pool,
        ):
            band_ts = []
            for i in range(3):
                bt = cpool.tile([P, P], f16, tag=f"band{i}")
                nc.scalar.dma_start(out=bt[:], in_=band_d[i])
                band_ts.append(bt)

            # Rotating x buffers with 7 leading and 3 trailing zero columns
            # (zeroed once; loads always write cols 7..7+W), so one scan of
            # length W+3 yields every output column incl. both edges.
            xsb = []
            for i in range(N_X):
                xt = cpool.tile([P, WB], f32, tag=f"xsb{i}")
                nc.gpsimd.memset(xt[:, 0:R], 0.0)
                nc.gpsimd.memset(xt[:, R + W : WB], 0.0)
                xsb.append(xt)

            # (band_idx, row_lo of the 128-row input slab, out_row, nvalid)
            tiles = []
            for img in range(PER_CORE):
                for t in range(NTILES):
                    o0 = t * MOUT
                    if t == 0:
                        lo = 0
                    elif t == NTILES - 1:
                        lo = H - P
                    else:
                        lo = o0 - PAD
                    nvalid = min(MOUT, H - o0)
                    tiles.append(
                        (0 if t == 0 else (2 if t == NTILES - 1 else 1),
                         img, lo, o0, nvalid)
                    )

            # Loads are emitted with a LOOKAHEAD lead over their consumers so
            # program order stays correct on the rotating buffers (load i+N_X
            # rewrites scan i's buffer, so it must be emitted AFTER scan i
            # and the lead must stay < N_X).  Full 128-partition loads only:
            # partition-offset HWDGE destinations fall off the descriptor
            # fast path (~6.6us/issue instead of 0.6).
            LOOKAHEAD = 8
            n_total = len(tiles)

            def emit_load(i):
                _, img, lo, _, _ = tiles[i]
                nc.sync.dma_start(
                    out=xsb[i % N_X][:, R : R + W],
                    in_=x_d[img, lo : lo + P, :],
                )

            for i in range(min(LOOKAHEAD, n_total)):
                emit_load(i)

            for i, (band_idx, img, lo, o0, nvalid) in enumerate(tiles):
                if i + LOOKAHEAD < n_total:
                    emit_load(i + LOOKAHEAD)
                x_t = xsb[i % N_X]

                # horizontal sliding 7-sum, one full-rate DVE instruction;
                # the scan state is fp32 internally and downcasts to fp16 on
                # write, so the 2-byte matmul (full-rate streaming, 1024-col
                # moving operand) gets its rhs with no extra cast op.
                h_t = hpool.tile([P, HB], f16)
                nc.vector._custom_dve(
                    boxsum7,
                    out=h_t[:, HOFF : HOFF + WIN],
                    in0=x_t[:, R : R + WIN],
                    in1=x_t[:, 0:WIN],
                )

                # vertical 7-sum: banded fp16 matmul -> 2D boxsum in PSUM
                # (2x 512-col MMs: a single MM's PSUM output is 1-bank max)
                v_ps = psum_pool.tile([MOUT, W], f32)
                bt = band_ts[band_idx]
                for j in range(2):
                    nc.tensor.matmul(
                        v_ps[:, j * 512 : (j + 1) * 512],
                        bt[:, 0:MOUT],
                        h_t[:, HOFF + PAD + j * 512 : HOFF + PAD + (j + 1) * 512],
                        start=True,
                        stop=True,
                    )

                # threshold straight from PSUM: mask = boxsum > 0 -> int8
                m_t = mpool.tile([P, W], mybir.dt.int8)
                nc.scalar.activation(
                    m_t[:MOUT, :], v_ps[:, :], sig, scale=SIG_SCALE,
                )

                # int8 SWDGE out
                nc.gpsimd.dma_start(
                    out=y_d[img, o0 : o0 + nvalid, :],
                    in_=m_t[0:nvalid, :],
                )

    nc.compile()
    return nc


_PROGRAM_CACHE = {}


def _get_program():
    if "nc" not in _PROGRAM_CACHE:
        _PROGRAM_CACHE["nc"] = _build_program()
    return _PROGRAM_CACHE["nc"]


def kernel(x, weight=None, **_unused):
    x = np.ascontiguousarray(np.asarray(x), dtype=np.float32)
    assert x.shape == (B, 1, H, W), x.shape
    xs = x.reshape(B, H, W)
    band = _band_matrices()

    nc = _get_program()
    in_maps = [
        {"x": np.ascontiguousarray(xs[c * PER_CORE : (c + 1) * PER_CORE]), "band": band}
        for c in range(NCORES)
    ]
    res = run_bass_kernel_spmd(nc, in_maps, core_ids=list(range(NCORES)))
    out = np.concatenate([r["y"] for r in res.results], axis=0)
    return out.reshape(B, 1, H, W).astype(np.int32)


# revision 25
# speedup vs baseline: 3.0721x; 1.0107x over previous
"""Trainium2 Bass kernel for nn_Dilate: 7x7 all-ones conv (same padding) -> (y > 0) int32 mask.

Input  x: (16, 1, 1024, 1024) float32, weight: (1, 1, 7, 7) ones (values unused).
Output:   (16, 1, 1024, 1024) int32 in {0, 1}.

Per core (pure batch data-parallel, 2 images/core on 8 cores), the 2D box
sum is separated HORIZONTAL-first so each engine does exactly one pass per
tile and the whole thing pipelines at the input-DMA roofline:

  - Row-tiles: 128 input rows (incl. 3+3 halo) -> 122 output rows, 9/image.
  - x loads via HWDGE (sync ring) into rotating [128, 7+W+3] SBUF buffers
    whose 7 leading + 3 trailing columns are zeroed once at startup.
  - Horizontal 7-tap sum in ONE custom-DVE instruction (registered at import
    into concourse.dve_ops.OPS): h = scan(ADD, Src0 - Src1) over the padded
    buffer = running sum of (x[t] - x[t-7]) = sliding 7-window sum.  The
    custom uop runs the recurrence at full rate (~1.2us/tile vs 2.3us for
    the stock tensor_tensor_scan, which routes its state backward through
    the pipe and halves throughput).
  - Vertical 7-tap sum on TensorE: banded ones matrix [128,122] as lhsT,
    fp32r matmul (PE rounds f32r internally; h is bitcast, no cast op),
    2x 512-col matmuls -> PSUM [122, 1024].
  - Threshold on ScalarE straight out of PSUM: sigmoid(1e8*boxsum) + round
    -to-nearest int8 cast (decision boundary exactly at boxsum=0).
  - int8 masks leave via GpSimd SWDGE; the host widens to int32.

Engine budget/tile: DVE ~1.21us, ACT ~1.0us, PE ~0.7-1.9us, DMA-in ~1.45us.
"""

import numpy as np

import concourse.bacc as bacc
import concourse.mybir as mybir
import concourse.dve_ops as dve_ops
from concourse.dve_spec import Spec, Src0, Src1, AluOp, scan, lower, _has_src1
from concourse.dve_uop import DveOpSpec
from concourse.tile import TileContext
from concourse.bass_utils import run_bass_kernel_spmd

B, H, W = 16, 1024, 1024
NCORES = 8
PER_CORE = B // NCORES  # 2 images per core
R = 7
PAD = R // 2  # 3
P = 128             # SBUF partitions per tile (input rows incl. halo)
MOUT = P - (R - 1)  # 122 output rows per tile
NTILES = -(-H // MOUT)  # 9 row tiles per image

WIN = W + PAD       # scan length: h col t = boxsum for output col j = t - 3
WB = R + W + PAD    # x tile width incl. 7 leading + 3 trailing zero cols
HOFF = 13           # h write offset so the matmul rhs (HOFF+PAD) is 32B-aligned
HB = HOFF + WIN     # h tile width

SIG_SCALE = 1.0e8   # pre-scale for the sigmoid threshold trick
N_X = 10            # rotating once-zero-padded x buffers (DMA prefetch depth)


def _register_boxsum7():
    """Register the custom DVE op (idempotent): out = cumsum(in0 - in1)."""
    name = "BOXSUM7_ANT"
    for op in dve_ops.OPS:
        if op.name == name:
            return op
    spec = Spec(
        body=scan(AluOp.ADD, Src0 - Src1),
        reference=lambda in0, in1, s0, s1, imm2: np.cumsum(
            in0.astype(np.float32) - in1.astype(np.float32), axis=-1
        ).astype(np.float32),
    )
    row = dve_ops._CUSTOM_DVE_ROW_BASE + len(dve_ops.OPS)
    assert row < 0x20, "custom-DVE row space exhausted"
    shas = {}
    for ver in ("v3", "v4"):
        s = DveOpSpec(name=name, opcode=row, uops=lower(spec, ver=ver),
                      rd1_en=_has_src1(spec))
        shas[ver] = s.sha(ver)
    op = dve_ops.DveOp(name, spec, subdim=False, uops_sha=shas)
    dve_ops.OPS.append(op)
    dve_ops._SUB_OPCODE_FOR_NAME[name] = row
    dve_ops.CUSTOM_DVE_SPECS[name] = spec
    return op


def _band_matrices() -> np.ndarray:
    """bands[0]: t=0 (partition p = image row p, top clamp);
    bands[1]: interior (partition p = row o0-3+p);
    bands[2]: last tile (partition p = row H-128+p, bottom clamp).
    band[k, m] = 1 iff output row m sums input partition k.
    Padded to 128 columns so the DMA moves 512 B/partition (line rate)."""
    bands = np.zeros((3, P, P), dtype=np.float16)
    for m in range(MOUT):
        bands[0, max(0, m - PAD) : m + PAD + 1, m] = 1.0
        bands[1, m : m + R, m] = 1.0
    # last tile: outputs start at row H-48 = partition 80
    for m in range(48):
        bands[2, 80 + m - PAD : min(80 + m + PAD + 1, P), m] = 1.0
    return bands


def _build_program():
    boxsum7 = _register_boxsum7()

    nc = bacc.Bacc("TRN2")
    x_d = nc.dram_tensor("x", [PER_CORE, H, W], mybir.dt.float32, kind="ExternalInput")
    band_d = nc.dram_tensor("band", [3, P, P], mybir.dt.float16, kind="ExternalInput")
    y_d = nc.dram_tensor("y", [PER_CORE, H, W], mybir.dt.int8, kind="ExternalOutput")

    sig = mybir.ActivationFunctionType.Sigmoid
    f16 = mybir.dt.float16
    f32 = mybir.dt.float32

    with TileContext(nc) as tc:
        with (
            tc.tile_pool(name="const", bufs=1) as cpool,
            tc.tile_pool(name="hbuf", bufs=6) as hpool,
            tc.tile_pool(name="mask", bufs=18) as mpool,
            tc.tile_pool(name="psum", bufs=4, space="PSUM") as psum_pool,
        ):
            band_ts = []
            for i in range(3):
                bt = cpool.tile([P, P], f16, tag=f"band{i}")
                nc.scalar.dma_start(out=bt[:], in_=band_d[i])
                band_ts.append(bt)

            # Rotating x buffers with 7 leading and 3 trailing zero columns
            # (zeroed once; loads always write cols 7..7+W), so one scan of
            # length W+3 yields every output column incl. both edges.
            xsb = []
            for i in range(N_X):
                xt = cpool.tile([P, WB], f32, tag=f"xsb{i}")
                nc.gpsimd.memset(xt[:, 0:R], 0.0)
                nc.gpsimd.memset(xt[:, R + W : WB], 0.0)
                xsb.append(xt)

            # (band_idx, row_lo of the 128-row input slab, out_row, nvalid)
            tiles = []
            for img in range(PER_CORE):
                for t in range(NTILES):
                    o0 = t * MOUT
                    if t == 0:
                        lo = 0
                    elif t == NTILES - 1:
                        lo = H - P
                    else:
                        lo = o0 - PAD
                    nvalid = min(MOUT, H - o0)
                    tiles.append(
                        (0 if t == 0 else (2 if t == NTILES - 1 else 1),
                         img, lo, o0, nvalid)
                    )

            # Loads are emitted with a LOOKAHEAD lead over their consumers so
            # program order stays correct on the rotating buffers (load i+N_X
            # rewrites scan i's buffer, so it must be emitted AFTER scan i
            # and the lead must stay < N_X).  Full 128-partition loads only:
            # partition-offset HWDGE destinations fall off the descriptor
            # fast path (~6.6us/issue instead of 0.6).
            LOOKAHEAD = 8
            n_total = len(tiles)

            def emit_load(i):
                _, img, lo, _, _ = tiles[i]
                nc.sync.dma_start(
                    out=xsb[i % N_X][:, R : R + W],
                    in_=x_d[img, lo : lo + P, :],
                )

            for i in range(min(LOOKAHEAD, n_total)):
                emit_load(i)

            for i, (band_idx, img, lo, o0, nvalid) in enumerate(tiles):
                if i + LOOKAHEAD < n_total:
                    emit_load(i + LOOKAHEAD)
                x_t = xsb[i % N_X]

                # horizontal sliding 7-sum, one full-rate DVE instruction;
                # the scan state is fp32 internally and downcasts to fp16 on
                # write, so the 2-byte matmul (full-rate streaming, 1024-col
                # moving operand) gets its rhs with no extra cast op.
                h_t = hpool.tile([P, HB], f16)
                nc.vector._custom_dve(
                    boxsum7,
                    out=h_t[:, HOFF : HOFF + WIN],
                    in0=x_t[:, R : R + WIN],
                    in1=x_t[:, 0:WIN],
                )

                # vertical 7-sum: banded fp16 matmul -> 2D boxsum in PSUM
                # (2x 512-col MMs: a single MM's PSUM output is 1-bank max)
                v_ps = psum_pool.tile([MOUT, W], f32)
                bt = band_ts[band_idx]
                for j in range(2):
                    nc.tensor.matmul(
                        v_ps[:, j * 512 : (j + 1) * 512],
                        bt[:, 0:MOUT],
                        h_t[:, HOFF + PAD + j * 512 : HOFF + PAD + (j + 1) * 512],
                        start=True,
                        stop=True,
                    )

                # threshold straight from PSUM: mask = boxsum > 0 -> int8
                m_t = mpool.tile([P, W], mybir.dt.int8)
                nc.scalar.activation(
                    m_t[:MOUT, :], v_ps[:, :], sig, scale=SIG_SCALE,
                )

                # int8 SWDGE out
                nc.gpsimd.dma_start(
                    out=y_d[img, o0 : o0 + nvalid, :],
                    in_=m_t[0:nvalid, :],
                )

    nc.compile()
    return nc


_PROGRAM_CACHE = {}


def _get_program():
    if "nc" not in _PROGRAM_CACHE:
        _PROGRAM_CACHE["nc"] = _build_program()
    return _PROGRAM_CACHE["nc"]


def kernel(x, weight=None, **_unused):
    x = np.ascontiguousarray(np.asarray(x), dtype=np.float32)
    assert x.shape == (B, 1, H, W), x.shape
    xs = x.reshape(B, H, W)
    band = _band_matrices()

    nc = _get_program()
    in_maps = [
        {"x": np.ascontiguousarray(xs[c * PER_CORE : (c + 1) * PER_CORE]), "band": band}
        for c in range(NCORES)
    ]
    res = run_bass_kernel_spmd(nc, in_maps, core_ids=list(range(NCORES)))
    out = np.concatenate([r["y"] for r in res.results], axis=0)
    return out.reshape(B, 1, H, W).astype(np.int32)
